# revision 1
# baseline (speedup 1.0000x reference)
"""LocalLinear (per-position 1D conv, K=8) Trainium2 Bass kernel.

Y[n, p] = sum_k X[n, p+k] * W[p, k, 0] + b[p, 0],  X right-padded by K-1.

Strategy: shard the position axis P across the 8 cores (2500 positions each,
with a 7-row halo). On the host, transpose X to X^T [P+7, N] and repack it
into per-chunk operand tiles of 128 rows: rows 0..cw+6 are X^T rows for the
chunk (cw<=120 output columns), row 127 is all-ones (carries the bias).
The per-position weights become a banded stationary matrix B [128, cw] per
chunk: B[j+k, j] = W[p0+j, k], B[127, j] = b[p0+j].  One fp32 matmul per
(chunk, 512-col half of N) computes Y^T for the chunk directly:
    out[j, n] = sum_q B[q, j] * rhs[q, n]
PE -> PSUM -> (DVE half0 / ACT half1 copies) -> SBUF -> DMA out Y^T.
Host transposes the gathered Y^T back to Y.
"""

import numpy as np

N = 1024
P = 20000
K = 8
NCORES = 8
PPC = P // NCORES  # positions per core
CW = 120  # output columns per chunk (CW + K - 1 = 127 <= 127, row 127 = bias)
CHUNKS = [(i * CW, min(CW, PPC - i * CW)) for i in range((PPC + CW - 1) // CW)]
NCH = len(CHUNKS)  # 21
NB = 4  # x (rhs) SBUF buffers
PB = 1  # PSUM buffers; >1 together with NB>1 and YB>1 races (HW sem timing)
YB = 4  # y SBUF buffers
HALF = 512

_CACHE = {}


def _build_bass():
    import concourse.bass as bass
    from concourse import mybir

    f32 = mybir.dt.float32
    nc = bass.Bass()
    rhs_d = nc.dram_tensor("rhs", [NCH, 128, N], f32, kind="ExternalInput")
    bmat_d = nc.dram_tensor("bmat", [128, NCH * CW], f32, kind="ExternalInput")
    yt_d = nc.dram_tensor("yt", [PPC, N], f32, kind="ExternalOutput")

    with (
        nc.sbuf_tensor("bmat_s", [128, NCH * CW], f32) as bmat_s,
        nc.sbuf_tensor("x_s", [128, NB * N], f32) as x_s,
        nc.sbuf_tensor("y_s", [128, YB * N], f32) as y_s,
        nc.psum_tensor("ps", [128, PB * N], f32) as ps,
        nc.semaphore("s_b") as s_b,
        nc.semaphore("s_in") as s_in,
        nc.semaphore("s_pe") as s_pe,
        nc.semaphore("s_dve") as s_dve,
        nc.semaphore("s_act") as s_act,
        nc.semaphore("s_out") as s_out,
        nc.Block() as block,
    ):

        @block.sync
        def _(sync):
            sync.dma_start(out=bmat_s[:], in_=bmat_d[:]).then_inc(s_b, 16)
            for c in range(NCH):
                if c >= NB:
                    # x slot free once PE finished chunk c-NB
                    sync.wait_ge(s_pe, c - NB + 1)
                xs = (c % NB) * N
                sync.dma_start(out=x_s[:, xs : xs + N], in_=rhs_d[c]).then_inc(
                    s_in, 16
                )

        @block.tensor
        def _(tensor):
            tensor.wait_ge(s_b, 16)
            for c in range(NCH):
                cs, cw = CHUNKS[c]
                tensor.wait_ge(s_in, 16 * (c + 1))
                if c >= PB:
                    tensor.wait_ge(s_dve, c - PB + 1)
                    tensor.wait_ge(s_act, c - PB + 1)
                xs = (c % NB) * N
                pp = (c % PB) * N
                lhsT = bmat_s[:, c * CW : c * CW + cw]
                tensor.matmul(
                    ps[0:cw, pp : pp + HALF],
                    lhsT,
                    x_s[:, xs : xs + HALF],
                    start=True,
                    stop=True,
                )
                tensor.matmul(
                    ps[0:cw, pp + HALF : pp + N],
                    lhsT,
                    x_s[:, xs + HALF : xs + N],
                    start=True,
                    stop=True,
                )
                # fp32 matmuls lower to 2 internal HW matmuls; an inc on the
                # matmul itself fires before the PSUM drain of the second
                # pass lands. Drain flushes the PE pipe before signalling.
                tensor.drain().then_inc(s_pe, 1)

        @block.vector
        def _(vector):
            for c in range(NCH):
                cs, cw = CHUNKS[c]
                vector.wait_ge(s_pe, c + 1)
                if c >= YB:
                    vector.wait_ge(s_out, 16 * (c - YB + 1))
                pp = (c % PB) * N
                ys = (c % YB) * N
                vector.tensor_copy(
                    y_s[0:cw, ys : ys + HALF], ps[0:cw, pp : pp + HALF]
                ).then_inc(s_dve, 1)

        @block.scalar
        def _(scalar):
            for c in range(NCH):
                cs, cw = CHUNKS[c]
                scalar.wait_ge(s_pe, c + 1)
                if c >= YB:
                    scalar.wait_ge(s_out, 16 * (c - YB + 1))
                pp = (c % PB) * N
                ys = (c % YB) * N
                scalar.copy(
                    y_s[0:cw, ys + HALF : ys + N], ps[0:cw, pp + HALF : pp + N]
                ).then_inc(s_act, 1)

        @block.gpsimd
        def _(g):
            for c in range(NCH):
                cs, cw = CHUNKS[c]
                g.wait_ge(s_dve, c + 1)
                g.wait_ge(s_act, c + 1)
                ys = (c % YB) * N
                g.dma_start(
                    out=yt_d[cs : cs + cw, :], in_=y_s[0:cw, ys : ys + N]
                ).then_inc(s_out, 16)

    return nc


def _prepare_inputs(X, W, b):
    """Host-side shard + repack: per-core rhs [NCH, 128, N] and bmat [128, NCH*CW]."""
    X = np.ascontiguousarray(X, dtype=np.float32)
    Ws = np.ascontiguousarray(W[:, :, 0], dtype=np.float32)  # [P, K]
    bs = np.ascontiguousarray(b[:, 0], dtype=np.float32)  # [P]

    XT = np.zeros((P + K - 1, N), np.float32)
    XT[:P] = X.T

    in_maps = []
    for i in range(NCORES):
        base = i * PPC
        rhs = np.zeros((NCH, 128, N), np.float32)
        bmat = np.zeros((128, NCH * CW), np.float32)
        for c, (cs, cw) in enumerate(CHUNKS):
            p0 = base + cs
            rhs[c, : cw + K - 1] = XT[p0 : p0 + cw + K - 1]
            rhs[c, 127] = 1.0
            j = np.arange(cw)
            for k in range(K):
                bmat[j + k, c * CW + j] = Ws[p0 + j, k]
            bmat[127, c * CW + j] = bs[p0 + j]
        in_maps.append({"rhs": rhs, "bmat": bmat})
    return in_maps


def _run(in_maps, trace=False):
    from concourse import bass_utils

    if "nc" not in _CACHE:
        _CACHE["nc"] = _build_bass()
    return bass_utils.run_bass_kernel_spmd(
        _CACHE["nc"], in_maps, core_ids=list(range(NCORES)), trace=trace
    )


def kernel(X, W, b):
    in_maps = _prepare_inputs(X, W, b)
    res = _run(in_maps)
    YT = np.concatenate([r["yt"] for r in res.results], axis=0)  # [P, N]
    return np.ascontiguousarray(YT.T)



# revision 5
# speedup vs baseline: 2.0037x; 2.0037x over previous
"""LocalLinear (per-position 1D conv, K=8) Trainium2 Bass kernel.

Y[n, p] = sum_k X[n, p+k] * W[p, k, 0] + b[p, 0],  X right-padded by K-1.

Strategy: shard the position axis P across the 8 cores (2500 positions each,
with a 7-row halo). The kernel is HBM-bandwidth bound (inputs + outputs are
~160MB vs ~2.9TB/s chip HBM), so all bulk traffic is fp16 with fp32 PSUM
accumulation (end-to-end max rel err ~5.5e-4, far inside the 2e-2 gate).

Per core, positions are processed in chunks of CW=121. One fused fp16 tile
per chunk holds both operands: cols 0..1023 are X^T rows p0..p0+cw+6
(cw+7 <= 128 partitions), cols 1024..1024+cw-1 are the banded stationary
matrix B with B[j+k, j] = W[p0+j, k]. One fp16 matmul per (chunk, 512-col
half of N) computes Y^T[j, n] = sum_q B[q, j] * X^T[p0+q, n] into fp32 PSUM.
The PSUM->SBUF drain casts to fp16 and adds the (fp32) bias b[p0+j] as a
per-partition scalar: DVE tensor_scalar_add for half 0, ACT activation
Identity-with-bias for half 1. gpsimd DMAs fp16 Y^T out; the host upcasts
and transposes back.
"""

import numpy as np

N = 1024
P = 20000
K = 8
NCORES = 8
PPC = P // NCORES  # positions per core
CW = 121  # output columns per chunk (CW + K - 1 = 128 partitions)
CHUNKS = [(i * CW, min(CW, PPC - i * CW)) for i in range((PPC + CW - 1) // CW)]
NCH = len(CHUNKS)  # 21, last chunk cw=80
XCOLS = N  # X^T columns per tile
TW = XCOLS + 128  # fused tile width (X cols + banded-W cols, 64B aligned)
NB = 5  # fused input tile SBUF buffers
PB = 2  # PSUM buffers (bank pairs)
YB = 4  # y SBUF buffers
HALF = 512

_CACHE = {}


def _build_bass():
    import concourse.bass as bass
    from concourse import mybir

    f16 = mybir.dt.float16
    f32 = mybir.dt.float32
    nc = bass.Bass()
    xin_d = nc.dram_tensor("xin", [NCH, 128, TW], f16, kind="ExternalInput")
    bvec_d = nc.dram_tensor("bvec", [128, NCH], f32, kind="ExternalInput")
    yt_d = nc.dram_tensor("yt", [PPC, N], f16, kind="ExternalOutput")

    with (
        nc.sbuf_tensor("bvec_s", [128, NCH], f32) as bvec_s,
        nc.sbuf_tensor("x_s", [128, NB * TW], f16) as x_s,
        nc.sbuf_tensor("y_s", [128, YB * N], f16) as y_s,
        nc.psum_tensor("ps", [128, PB * N], f32) as ps,
        nc.semaphore("s_b") as s_b,
        nc.semaphore("s_in") as s_in,
        nc.semaphore("s_pe") as s_pe,
        nc.semaphore("s_dve") as s_dve,
        nc.semaphore("s_act") as s_act,
        nc.semaphore("s_out") as s_out,
        nc.Block() as block,
    ):

        @block.sync
        def _(sync):
            sync.dma_start(out=bvec_s[:], in_=bvec_d[:]).then_inc(s_b, 16)
            for c in range(NCH):
                cs, cw = CHUNKS[c]
                rows = cw + K - 1
                if c >= NB:
                    # x slot free once both matmul halves of chunk c-NB ran
                    sync.wait_ge(s_pe, 2 * (c - NB) + 2)
                xs = (c % NB) * TW
                sync.dma_start(
                    out=x_s[0:rows, xs : xs + TW], in_=xin_d[c, 0:rows]
                ).then_inc(s_in, 16)
            # Sentinel: one extra in-queue DMA so the PE can wait for
            # "chunk c+1's DMA done" even at c = NCH-1 (see tensor block).
            sync.dma_start(out=bvec_s[:], in_=bvec_d[:]).then_inc(s_in, 16)

        @block.tensor
        def _(tensor):
            for c in range(NCH):
                cs, cw = CHUNKS[c]
                rows = cw + K - 1
                # Wait for chunk c+1's DMA: the completion inc of chunk c's
                # own DMA can fire before its last writes are visible to PE
                # (observed as partition-band corruption in matmul half 0).
                # Queue completions are in order, so c+1 done => c landed
                # ~1.7us earlier (one transfer + sem propagation).
                tensor.wait_ge(s_in, 16 * (c + 2))
                if c >= PB:
                    tensor.wait_ge(s_dve, c - PB + 1)
                    tensor.wait_ge(s_act, c - PB + 1)
                xs = (c % NB) * TW
                pp = (c % PB) * N
                lhsT = x_s[0:rows, xs + XCOLS : xs + XCOLS + cw]
                tensor.matmul(
                    ps[0:cw, pp : pp + HALF],
                    lhsT,
                    x_s[0:rows, xs : xs + HALF],
                    start=True,
                    stop=True,
                )
                # drain per half: signals after the PSUM writes land, and
                # lets DVE start on half 0 while PE runs half 1
                tensor.drain().then_inc(s_pe, 1)
                tensor.matmul(
                    ps[0:cw, pp + HALF : pp + N],
                    lhsT,
                    x_s[0:rows, xs + HALF : xs + XCOLS],
                    start=True,
                    stop=True,
                )
                tensor.drain().then_inc(s_pe, 1)

        @block.vector
        def _(vector):
            vector.wait_ge(s_b, 16)
            for c in range(NCH):
                cs, cw = CHUNKS[c]
                vector.wait_ge(s_pe, 2 * c + 1)
                if c >= YB:
                    vector.wait_ge(s_out, 16 * (c - YB + 1))
                pp = (c % PB) * N
                ys = (c % YB) * N
                vector.tensor_scalar_add(
                    y_s[0:cw, ys : ys + HALF],
                    ps[0:cw, pp : pp + HALF],
                    bvec_s[0:cw, c : c + 1],
                ).then_inc(s_dve, 1)

        @block.scalar
        def _(scalar):
            scalar.wait_ge(s_b, 16)
            for c in range(NCH):
                cs, cw = CHUNKS[c]
                scalar.wait_ge(s_pe, 2 * c + 2)
                if c >= YB:
                    scalar.wait_ge(s_out, 16 * (c - YB + 1))
                pp = (c % PB) * N
                ys = (c % YB) * N
                scalar.add(
                    y_s[0:cw, ys + HALF : ys + N],
                    ps[0:cw, pp + HALF : pp + N],
                    bvec_s[0:cw, c : c + 1],
                ).then_inc(s_act, 1)

        @block.gpsimd
        def _(g):
            for c in range(NCH):
                cs, cw = CHUNKS[c]
                g.wait_ge(s_dve, c + 1)
                g.wait_ge(s_act, c + 1)
                ys = (c % YB) * N
                g.dma_start(
                    out=yt_d[cs : cs + cw, :], in_=y_s[0:cw, ys : ys + N]
                ).then_inc(s_out, 16)

    return nc


def _prepare_inputs(X, W, b):
    """Host-side shard + repack: fused fp16 tiles [NCH, 128, TW] per core."""
    Xh = np.ascontiguousarray(X, dtype=np.float32).astype(np.float16)
    Wh = np.ascontiguousarray(W[:, :, 0], dtype=np.float32).astype(np.float16)
    bs = np.ascontiguousarray(b[:, 0], dtype=np.float32)  # [P]

    XT = np.zeros((P + K - 1, N), np.float16)
    XT[:P] = Xh.T

    in_maps = []
    for i in range(NCORES):
        base = i * PPC
        xin = np.zeros((NCH, 128, TW), np.float16)
        bvec = np.zeros((128, NCH), np.float32)
        for c, (cs, cw) in enumerate(CHUNKS):
            p0 = base + cs
            rows = cw + K - 1
            xin[c, :rows, :XCOLS] = XT[p0 : p0 + rows]
            j = np.arange(cw)
            for k in range(K):
                xin[c, j + k, XCOLS + j] = Wh[p0 + j, k]
            bvec[:cw, c] = bs[p0 : p0 + cw]
        in_maps.append({"xin": xin, "bvec": bvec})
    return in_maps


def _run(in_maps, trace=False):
    from concourse import bass_utils

    if "nc" not in _CACHE:
        _CACHE["nc"] = _build_bass()
    return bass_utils.run_bass_kernel_spmd(
        _CACHE["nc"], in_maps, core_ids=list(range(NCORES)), trace=trace
    )


def kernel(X, W, b):
    in_maps = _prepare_inputs(X, W, b)
    res = _run(in_maps)
    YT = np.concatenate([r["yt"] for r in res.results], axis=0)  # [P, N] f16
    return np.ascontiguousarray(YT.T.astype(np.float32))


# revision 7
# speedup vs baseline: 2.0097x; 1.0030x over previous
"""LocalLinear (per-position 1D conv, K=8) Trainium2 Bass kernel.

Y[n, p] = sum_k X[n, p+k] * W[p, k, 0] + b[p, 0],  X right-padded by K-1.

Strategy: shard the position axis P across the 8 cores (2500 positions each,
with a 7-row halo). The kernel is HBM-bandwidth bound (inputs + outputs are
~160MB vs ~2.9TB/s chip HBM), so all bulk traffic is fp16 with fp32 PSUM
accumulation (end-to-end max rel err ~5.5e-4, far inside the 2e-2 gate).

Per core, positions are processed in chunks of CW=121. One fused fp16 tile
per chunk holds both operands: cols 0..1023 are X^T rows p0..p0+cw+6
(cw+7 <= 128 partitions), cols 1024..1024+cw-1 are the banded stationary
matrix B with B[j+k, j] = W[p0+j, k]. One fp16 matmul per (chunk, 512-col
half of N) computes Y^T[j, n] = sum_q B[q, j] * X^T[p0+q, n] into fp32 PSUM.
The PSUM->SBUF drain casts to fp16 and adds the (fp32) bias b[p0+j] as a
per-partition scalar: DVE tensor_scalar_add for half 0, ACT activation
Identity-with-bias for half 1. gpsimd DMAs fp16 Y^T out; the host upcasts
and transposes back.
"""

import numpy as np

N = 1024
P = 20000
K = 8
NCORES = 8
PPC = P // NCORES  # positions per core
CW = 121  # output columns per chunk (CW + K - 1 = 128 partitions)
CHUNKS = [(i * CW, min(CW, PPC - i * CW)) for i in range((PPC + CW - 1) // CW)]
NCH = len(CHUNKS)  # 21, last chunk cw=80
XCOLS = N  # X^T columns per tile
TW = XCOLS + 128  # fused tile width (X cols + banded-W cols, 64B aligned)
NB = 8  # fused input tile SBUF buffers
PB = 2  # PSUM buffers (bank pairs)
YB = 6  # y SBUF buffers
HALF = 512

_CACHE = {}


def _build_bass():
    import concourse.bass as bass
    from concourse import mybir

    f16 = mybir.dt.float16
    f32 = mybir.dt.float32
    nc = bass.Bass()
    xin_d = nc.dram_tensor("xin", [NCH, 128, TW], f16, kind="ExternalInput")
    bvec_d = nc.dram_tensor("bvec", [128, NCH], f32, kind="ExternalInput")
    yt_d = nc.dram_tensor("yt", [PPC, N], f16, kind="ExternalOutput")

    with (
        nc.sbuf_tensor("bvec_s", [128, NCH], f32) as bvec_s,
        nc.sbuf_tensor("x_s", [128, NB * TW], f16) as x_s,
        nc.sbuf_tensor("y_s", [128, YB * N], f16) as y_s,
        nc.psum_tensor("ps", [128, PB * N], f32) as ps,
        nc.semaphore("s_b") as s_b,
        nc.semaphore("s_in") as s_in,
        nc.semaphore("s_pe") as s_pe,
        nc.semaphore("s_dve") as s_dve,
        nc.semaphore("s_act") as s_act,
        nc.semaphore("s_out") as s_out,
        nc.Block() as block,
    ):

        @block.sync
        def _(sync):
            sync.dma_start(out=bvec_s[:], in_=bvec_d[:]).then_inc(s_b, 16)
            for c in range(NCH):
                cs, cw = CHUNKS[c]
                rows = cw + K - 1
                if c >= NB:
                    # x slot free once both matmul halves of chunk c-NB ran
                    sync.wait_ge(s_pe, 2 * (c - NB) + 2)
                xs = (c % NB) * TW
                w = XCOLS + cw  # skip unused pad columns
                sync.dma_start(
                    out=x_s[0:rows, xs : xs + w], in_=xin_d[c, 0:rows, 0:w]
                ).then_inc(s_in, 16)
            # Sentinel: one extra in-queue DMA so the PE can wait for
            # "chunk c+1's DMA done" even at c = NCH-1 (see tensor block).
            sync.dma_start(out=bvec_s[:], in_=bvec_d[:]).then_inc(s_in, 16)

        @block.tensor
        def _(tensor):
            for c in range(NCH):
                cs, cw = CHUNKS[c]
                rows = cw + K - 1
                # Wait for chunk c+1's DMA: the completion inc of chunk c's
                # own DMA can fire before its last writes are visible to PE
                # (observed as partition-band corruption in matmul half 0).
                # Queue completions are in order, so c+1 done => c landed
                # ~1.7us earlier (one transfer + sem propagation).
                tensor.wait_ge(s_in, 16 * (c + 2))
                if c >= PB:
                    tensor.wait_ge(s_dve, c - PB + 1)
                    tensor.wait_ge(s_act, c - PB + 1)
                xs = (c % NB) * TW
                pp = (c % PB) * N
                lhsT = x_s[0:rows, xs + XCOLS : xs + XCOLS + cw]
                tensor.matmul(
                    ps[0:cw, pp : pp + HALF],
                    lhsT,
                    x_s[0:rows, xs : xs + HALF],
                    start=True,
                    stop=True,
                )
                # drain per half: signals after the PSUM writes land, and
                # lets DVE start on half 0 while PE runs half 1
                tensor.drain().then_inc(s_pe, 1)
                tensor.matmul(
                    ps[0:cw, pp + HALF : pp + N],
                    lhsT,
                    x_s[0:rows, xs + HALF : xs + XCOLS],
                    start=True,
                    stop=True,
                )
                tensor.drain().then_inc(s_pe, 1)

        @block.vector
        def _(vector):
            vector.wait_ge(s_b, 16)
            for c in range(NCH):
                cs, cw = CHUNKS[c]
                vector.wait_ge(s_pe, 2 * c + 1)
                if c >= YB:
                    vector.wait_ge(s_out, 16 * (c - YB + 1))
                pp = (c % PB) * N
                ys = (c % YB) * N
                vector.tensor_scalar_add(
                    y_s[0:cw, ys : ys + HALF],
                    ps[0:cw, pp : pp + HALF],
                    bvec_s[0:cw, c : c + 1],
                ).then_inc(s_dve, 1)

        @block.scalar
        def _(scalar):
            scalar.wait_ge(s_b, 16)
            for c in range(NCH):
                cs, cw = CHUNKS[c]
                scalar.wait_ge(s_pe, 2 * c + 2)
                if c >= YB:
                    scalar.wait_ge(s_out, 16 * (c - YB + 1))
                pp = (c % PB) * N
                ys = (c % YB) * N
                scalar.add(
                    y_s[0:cw, ys + HALF : ys + N],
                    ps[0:cw, pp + HALF : pp + N],
                    bvec_s[0:cw, c : c + 1],
                ).then_inc(s_act, 1)

        @block.gpsimd
        def _(g):
            for c in range(NCH):
                cs, cw = CHUNKS[c]
                g.wait_ge(s_dve, c + 1)
                g.wait_ge(s_act, c + 1)
                ys = (c % YB) * N
                g.dma_start(
                    out=yt_d[cs : cs + cw, :], in_=y_s[0:cw, ys : ys + N]
                ).then_inc(s_out, 16)

    return nc


def _prepare_inputs(X, W, b):
    """Host-side shard + repack: fused fp16 tiles [NCH, 128, TW] per core."""
    Xh = np.ascontiguousarray(X, dtype=np.float32).astype(np.float16)
    Wh = np.ascontiguousarray(W[:, :, 0], dtype=np.float32).astype(np.float16)
    bs = np.ascontiguousarray(b[:, 0], dtype=np.float32)  # [P]

    XT = np.zeros((P + K - 1, N), np.float16)
    XT[:P] = Xh.T

    in_maps = []
    for i in range(NCORES):
        base = i * PPC
        xin = np.zeros((NCH, 128, TW), np.float16)
        bvec = np.zeros((128, NCH), np.float32)
        for c, (cs, cw) in enumerate(CHUNKS):
            p0 = base + cs
            rows = cw + K - 1
            xin[c, :rows, :XCOLS] = XT[p0 : p0 + rows]
            j = np.arange(cw)
            for k in range(K):
                xin[c, j + k, XCOLS + j] = Wh[p0 + j, k]
            bvec[:cw, c] = bs[p0 : p0 + cw]
        in_maps.append({"xin": xin, "bvec": bvec})
    return in_maps


def _run(in_maps, trace=False):
    from concourse import bass_utils

    if "nc" not in _CACHE:
        _CACHE["nc"] = _build_bass()
    return bass_utils.run_bass_kernel_spmd(
        _CACHE["nc"], in_maps, core_ids=list(range(NCORES)), trace=trace
    )


def kernel(X, W, b):
    in_maps = _prepare_inputs(X, W, b)
    res = _run(in_maps)
    YT = np.concatenate([r["yt"] for r in res.results], axis=0)  # [P, N] f16
    return np.ascontiguousarray(YT.T.astype(np.float32))
